# revision 24
# baseline (speedup 1.0000x reference)
"""Trainium2 Bass kernel for nn_DiscretePixelCNN.

Computes, for each image: sum over pixels of log p(sample) under a 6-layer
masked PixelCNN (7x7 convs, 64 hidden channels, K=2 discrete values),
returning [B, 1].

Strategy (data-parallel over batch, 4 images per core, 8 cores):
  - Padded canvas layout per core: 4 images stacked vertically, each row
    stored with 3+3 zero pad cols (row stride S=70), 3 zero pad rows above
    each image.  All shifted conv reads then become flat offsets.
  - Activations h stored as [128, NPX]: partitions 0:64 = h, partitions
    64:128 = h shifted one row down (h2[p] = h[p+S]).  This gives K=128
    matmul contraction (2 kernel rows x 64 channels per instruction).
  - Each masked-conv matmul instruction covers a 2x2 kernel patch via a
    [128,128] lhsT:  A00=(ky,kx)  A10=(ky+1,kx)  A01=(ky,kx+1)
    A11=(ky+1,kx+1); the M halves accumulate into psum[0:64] / psum[64:128]
    and are combined as out[u] = psumA[u] + psumB[u+1].
  - Layer 1 (mask A, 1 input channel) uses a host-prepared 24-partition
    im2col canvas (one partition per mask-A tap).
  - Output head: d = (w_out[1]-w_out[0]) . h per pixel, accumulated into a
    [40, 490] psum (one partition per chunk via sliding-window lhsT);
    logp_pixel = -softplus(sigma * (d + db)), sigma = 1-2*sample (host
    prepared); reduce per chunk then per image with a tiny matmul.
"""
import os
import sys

import numpy as np

for _p in ("/opt/trn_rl_repo", "/root/.axon_site/_ro/trn_rl_repo"):
    if os.path.isdir(_p) and _p not in sys.path:
        sys.path.insert(0, _p)

import concourse.bass as bass
from concourse import bacc
import concourse.mybir as mybir
import concourse.tile as tile
from concourse.bass_utils import run_bass_kernel_spmd

# ---- problem constants (hardcoded; kernel.py must be self-contained) ----
B, C, H, W = 32, 1, 64, 64
KVALS = 2
HID = 64
KS = 7
NL = 6
NCORE = 8
BPC = B // NCORE        # images per core

S = 70                  # canvas row stride (3 + 64 + 3)
COL0 = 3
# per-image canvas tile: [1 margin row][3 pad rows][64 real rows] = 68 rows
ROW0 = 4 * S            # 280: canvas px of (row 0, col 0-incl-pad)
NPX = 68 * S            # 4760
MAXN = 7 * S            # 490

DT_MM = os.environ.get("PIXELCNN_DT", "f32r")   # f32r | bf16 | f32

# conv chunks within one image tile: 9 x 7-row + 1 x 1-row
CH = [(ROW0 + c * 7 * S, 7, 7 * S) for c in range(9)] + [(ROW0 + 63 * S, 1, S)]
# output head: uniform 4-row chunks, 16 per image; partition = 16*img + c
OUT_N = 4 * S           # 280
OUT_CH = [ROW0 + c * 4 * S for c in range(16)]


def _spatial_mask(kind):
    m = np.zeros((KS, KS), np.float32)
    c = KS // 2
    m[:c, :] = 1.0
    m[c, :c] = 1.0
    if kind == 'B':
        m[c, c] = 1.0
    return m


MASK_A = _spatial_mask('A')
MASK_B = _spatial_mask('B')
TAPS_A = [(ky, kx) for ky in range(KS) for kx in range(KS) if MASK_A[ky, kx]]
PATCHES_B = [(ky, kx) for ky in (0, 2) for kx in (0, 2, 4, 6)]


def _d_off(ky, kx):
    return (ky - 3) * S + (kx - 3)


# ---------------------------------------------------------------- host prep
def _pack_weights(ws, bs, w_out, b_out, np_dt):
    """Returns dict of host-packed weight arrays shared by all cores."""
    ws = [np.asarray(w, np.float32) for w in ws]
    bs = [np.asarray(b, np.float32) for b in bs]
    w_out = np.asarray(w_out, np.float32)[:, :, 0, 0]   # [2, HID]
    b_out = np.asarray(b_out, np.float32)
    dw = w_out[1] - w_out[0]
    db = float(b_out[1] - b_out[0])

    # mask-B layers: [128, 5*8*128] in SBUF layout [k, (li, patch, m)]
    wB = np.zeros((128, (NL - 1) * 8 * 128), np.float32)
    for li in range(NL - 1):
        w = ws[li + 1] * MASK_B[None, None]
        for pi, (ky, kx) in enumerate(PATCHES_B):
            col = (li * 8 + pi) * 128
            blk = np.zeros((128, 128), np.float32)
            for (dk, dm, kyy, kxx) in ((0, 0, ky, kx), (64, 0, ky + 1, kx),
                                       (0, 64, ky, kx + 1),
                                       (64, 64, ky + 1, kx + 1)):
                if kyy < KS and kxx < KS and MASK_B[kyy, kxx]:
                    blk[dk:dk + 64, dm:dm + 64] = w[:, :, kyy, kxx].T
            wB[:, col:col + 128] = blk

    w1 = np.stack([ws[0][:, 0, ky, kx] for (ky, kx) in TAPS_A], 0)  # [24, 64]

    wout = np.zeros((64, 129), np.float32)
    wout[:, 64] = dw

    # spl holds log(sigmoid(-x)) = -softplus(x) per pixel, so sum with +1
    wsel = np.zeros((64, 4), np.float32)
    for i in range(BPC):
        wsel[16 * i:16 * i + 16, i] = 1.0

    biasmat = np.stack(bs, 1)   # [64, NL]

    return {
        "wB": wB.astype(np_dt), "w1": w1.astype(np_dt),
        "wout": wout.astype(np_dt), "wsel": wsel,
        "biasmat": biasmat, "db": db,
    }


def _per_core_inputs(sample, np_dt):
    """sample [B,1,H,W] fp32 -> list of dicts with imc + sigma per core."""
    sample = np.asarray(sample, np.float32).reshape(B, H, W)
    maps = []
    for core in range(NCORE):
        smp = sample[core * BPC:(core + 1) * BPC]
        # per-image im2col canvases: part t = sample shifted by -d_t
        imc = np.zeros((BPC, 24, NPX + 300), np.float32)
        for t, (ky, kx) in enumerate(TAPS_A):
            d = _d_off(ky, kx)
            base = ROW0 + COL0 - d
            for i in range(BPC):
                for y in range(H):
                    imc[i, t, base + y * S: base + y * S + W] = smp[i, y]
        sigma = np.ones((64, OUT_N), np.float32)
        for p in range(64):
            i, c = p // 16, p % 16
            for r in range(4):
                sigma[p, r * S + COL0: r * S + COL0 + W] = \
                    1.0 - 2.0 * smp[i, 4 * c + r]
        maps.append({"imc": np.ascontiguousarray(imc[:, :, :NPX]).astype(np_dt),
                     "sigma": sigma})
    return maps


# ---------------------------------------------------------------- bass build
def _build_bass(db, mm_dt_name):
    f32 = mybir.dt.float32
    if mm_dt_name == "bf16":
        store_dt = mybir.dt.bfloat16
    elif mm_dt_name == "f32":
        store_dt = f32
    else:
        # float32r end-to-end: walrus requires producers of fp32r matmul
        # inputs to declare the f32r dtype (bitcasts fail BIR verification)
        store_dt = mybir.dt.float32r

    def mm(ap):
        return ap

    # Bacc (not raw Bass): its finalize() runs generate_event_semaphores,
    # which splits multi-wait sync onto event sems (HW allows 1 wait/inst)
    nc = bacc.Bacc("TRN2", target_bir_lowering=False)
    d_imc = nc.declare_dram_parameter("imc", [BPC, 24, NPX], store_dt, isOutput=False)
    d_wB = nc.declare_dram_parameter("wB", [128, (NL - 1) * 8 * 128], store_dt, isOutput=False)
    d_w1 = nc.declare_dram_parameter("w1", [24, 64], store_dt, isOutput=False)
    d_wout = nc.declare_dram_parameter("wout", [64, 129], store_dt, isOutput=False)
    d_wsel = nc.declare_dram_parameter("wsel", [64, 4], f32, isOutput=False)
    d_bias = nc.declare_dram_parameter("biasmat", [64, NL], f32, isOutput=False)
    d_sigma = nc.declare_dram_parameter("sigma", [64, OUT_N], f32, isOutput=False)
    d_out = nc.declare_dram_parameter("out", [BPC, 1], f32, isOutput=True)

    AF = mybir.ActivationFunctionType
    ALU = mybir.AluOpType
    AX = mybir.AxisListType

    with tile.TileContext(nc) as tc:
        with (
            tc.tile_pool(name="cpA", bufs=1) as cpA,
            tc.tile_pool(name="cpB", bufs=1) as cpB,
            tc.tile_pool(name="wp", bufs=1) as wp,
            tc.tile_pool(name="small", bufs=1) as sp_pool,
            tc.tile_pool(name="tmp", bufs=3) as tmp_pool,
            tc.tile_pool(name="psB", bufs=4, space="PSUM") as psB_pool,
            tc.tile_pool(name="ps1", bufs=2, space="PSUM") as ps1_pool,
            tc.tile_pool(name="psd", bufs=1, space="PSUM") as psd_pool,
        ):
            qeng = [nc.gpsimd, nc.sync, nc.scalar]

            # ---- layer-1 inputs first, so PE can start ASAP ----
            imc_t = []
            for i in range(BPC):
                t = cpA.tile([24, NPX], store_dt, tag=f"cA{i}")
                for q in range(3):
                    eng = qeng[(i * 3 + q) % 3]
                    eng.dma_start(t[8 * q:8 * q + 8, :], d_imc[i, 8 * q:8 * q + 8, :])
                imc_t.append(t)
            w1_t = wp.tile([24, 64], store_dt, tag="w1")
            nc.gpsimd.dma_start(w1_t[:, :], d_w1[:, :])
            bias_t = wp.tile([64, NL], f32, tag="bias")
            nc.gpsimd.dma_start(bias_t[:, :], d_bias[:, :])
            # h1 slots: pads must be zero before layer-1 combine writes
            m1 = []
            for i in range(BPC):
                t = cpB.tile([128, NPX], store_dt, tag=f"cB{i}")
                eng = nc.vector if i % 2 == 0 else nc.gpsimd
                eng.memset(t[:, :].bitcast(mybir.dt.uint32), 0)
                m1.append(t)

            # ---- remaining weights / consts (per-layer slices, spread) ----
            wB_t = wp.tile([128, (NL - 1) * 8 * 128], store_dt, tag="wB")
            for li in range(NL - 1):
                sl = bass.ts(li, 8 * 128)
                qeng[(1 + li) % 3].dma_start(wB_t[:, sl], d_wB[:, sl])
            wout_t = wp.tile([64, 129], store_dt, tag="wout")
            nc.sync.dma_start(wout_t[:, :], d_wout[:, :])
            wsel_t = wp.tile([64, 4], f32, tag="wsel")
            nc.sync.dma_start(wsel_t[:, :], d_wsel[:, :])
            sigma_t = wp.tile([64, OUT_N], f32, tag="sigma")
            nc.sync.dma_start(sigma_t[:, :], d_sigma[:, :])

            def views(t, parts, base, n_rows, shift):
                """[64, n_rows, 64] view of canvas t at pixel rows of a chunk."""
                lo = base + shift
                v = t[parts[0]:parts[1], lo:lo + n_rows * S]
                return v.rearrange("p (r c) -> p r c", c=S)[:, :, COL0:COL0 + W]

            def ps_view(ps, parts, n_rows, shift):
                v = ps[parts[0]:parts[1], shift:shift + n_rows * S]
                return v.rearrange("p (r c) -> p r c", c=S)[:, :, COL0:COL0 + W]

            # ---- layer 1 (mask A) ----
            h_cur = m1
            b0 = bias_t[:, 0:1]
            for i in range(BPC):
                for (cbase, n_rows, N) in CH:
                    ps = ps1_pool.tile([64, 512], f32, tag="ps1")
                    nc.tensor.matmul(ps[:, :N], mm(w1_t[:, :]),
                                     mm(imc_t[i][:, cbase:cbase + N]),
                                     start=True, stop=True)
                    nc.scalar.activation(views(h_cur[i], (0, 64), cbase, n_rows, 0),
                                         ps_view(ps, (0, 64), n_rows, 0),
                                         AF.Relu, bias=b0)
                    nc.vector.tensor_scalar(
                        views(h_cur[i], (64, 128), cbase, n_rows, -S),
                        ps_view(ps, (0, 64), n_rows, 0),
                        b0, 0.0, op0=ALU.add, op1=ALU.max)

            # ---- layers 2..NL (mask B) ----
            for li in range(NL - 1):
                pool, ctag = (cpA, "cA") if li % 2 == 0 else (cpB, "cB")
                h_nxt = []
                for i in range(BPC):
                    t = pool.tile([128, NPX], store_dt, tag=f"{ctag}{i}")
                    if li == 0:
                        # slot held im2col and was never fully zeroed: zero
                        # before combine writes (gpsimd, during layer-1 tail)
                        nc.gpsimd.memset(t[:, :].bitcast(mybir.dt.uint32), 0)
                    h_nxt.append(t)
                bl = bias_t[:, li + 1:li + 2]
                for i in range(BPC):
                    for (cbase, n_rows, N) in CH:
                        ps = psB_pool.tile([128, 512], f32, tag="psB")
                        for pi, (ky, kx) in enumerate(PATCHES_B):
                            o = cbase + _d_off(ky, kx)
                            nc.tensor.matmul(
                                ps[:, :N],
                                mm(wB_t[:, (li * 8 + pi) * 128:(li * 8 + pi) * 128 + 128]),
                                mm(h_cur[i][:, o:o + N]),
                                start=(pi == 0), stop=(pi == 7))
                        t1 = tmp_pool.tile([64, 7 * W], f32, tag="t1")
                        t1v = t1[:, :n_rows * W].rearrange("p (r c) -> p r c", c=W)
                        # t1 = psumA + bias  (DVE; one PSUM input per inst)
                        nc.vector.tensor_scalar(
                            t1v, ps_view(ps, (0, 64), n_rows, 0), bl, None,
                            op0=ALU.add)
                        tmp = tmp_pool.tile([64, 7 * W], f32, tag="tmp")
                        tview = tmp[:, :n_rows * W].rearrange("p (r c) -> p r c", c=W)
                        # tmp = psumB_shifted + t1
                        nc.vector.scalar_tensor_tensor(
                            tview, ps_view(ps, (64, 128), n_rows, 1), 0.0, t1v,
                            op0=ALU.add, op1=ALU.add)
                        nc.scalar.activation(
                            views(h_nxt[i], (0, 64), cbase, n_rows, 0),
                            tview, AF.Relu)
                        nc.gpsimd.tensor_scalar(
                            views(h_nxt[i], (64, 128), cbase, n_rows, -S),
                            tview, 0.0, None, op0=ALU.max)
                h_cur = h_nxt

            # ---- output head: d per 4-row chunk into psum [64, 512] ----
            dall = psd_pool.tile([64, 512], f32, tag="dall")
            for p in range(64):
                i, c = p // 16, p % 16
                cbase = OUT_CH[c]
                nc.tensor.matmul(dall[:, :OUT_N], mm(wout_t[:, 64 - p:128 - p]),
                                 mm(h_cur[i][0:64, cbase:cbase + OUT_N]),
                                 start=(p == 0), stop=(p == 63))

            sd_t = sp_pool.tile([64, OUT_N], f32, tag="sd")
            nc.vector.scalar_tensor_tensor(sd_t[:, :], dall[:, :OUT_N], float(db),
                                           sigma_t[:, :], op0=ALU.add,
                                           op1=ALU.mult)
            # -softplus(x) = log(sigmoid(-x)); Softplus has no ACT func set
            sg_t = sp_pool.tile([64, OUT_N], f32, tag="sg")
            nc.scalar.activation(sg_t[:, :], sd_t[:, :], AF.Sigmoid, scale=-1.0)
            spl_t = sp_pool.tile([64, OUT_N], f32, tag="spl")
            nc.scalar.activation(spl_t[:, :], sg_t[:, :], AF.Ln)

            sums_t = sp_pool.tile([64, 1], f32, tag="sums")
            allv = spl_t[:, :].rearrange("p (r c) -> p r c", c=S)[:, :, COL0:COL0 + W]
            nc.vector.tensor_reduce(sums_t[:, 0:1], allv, axis=AX.XY, op=ALU.add)

            fin = psd_pool.tile([4, 1], f32, tag="fin")
            nc.tensor.matmul(fin[0:4, 0:1], wsel_t[:, :], sums_t[:, 0:1],
                             start=True, stop=True)
            out_sb = sp_pool.tile([4, 1], f32, tag="outsb")
            nc.scalar.copy(out_sb[:, :], fin[0:4, 0:1])
            nc.gpsimd.dma_start(d_out[:, :], out_sb[:, :])

    return nc


_CACHE = {}


def _get_nc(db):
    key = (DT_MM, round(db, 9))
    if key not in _CACHE:
        nc = _build_bass(db, DT_MM)
        nc.finalize()   # Bacc.compile(): event-sem split, reg alloc, codegen
        _CACHE[key] = nc
    return _CACHE[key]


def kernel(sample, ws, bs, w_out, b_out):
    np_dt = np.float32
    if DT_MM == "bf16":
        import ml_dtypes
        np_dt = ml_dtypes.bfloat16

    packed = _pack_weights(ws, bs, w_out, b_out, np_dt)
    core_maps = _per_core_inputs(sample, np_dt)
    shared = {k: packed[k] for k in ("wB", "w1", "wout", "wsel", "biasmat")}
    in_maps = [dict(shared, **m) for m in core_maps]

    nc = _get_nc(packed["db"])
    trace = bool(int(os.environ.get("PIXELCNN_TRACE", "0")))
    res = run_bass_kernel_spmd(nc, in_maps, list(range(NCORE)), trace=trace)
    if trace and res.exec_time_ns is not None:
        print(f"HW exec time: {res.exec_time_ns} ns")
        if res.mean_exec_time_ns is not None:
            print(f"HW mean exec time: {res.mean_exec_time_ns} ns")
    out = np.concatenate([np.asarray(r["out"], np.float32) for r in res.results], 0)
    return out


# revision 25
# speedup vs baseline: 2.9946x; 2.9946x over previous
"""Trainium2 Bass kernel for nn_DiscretePixelCNN.

Computes, for each image: sum over pixels of log p(sample) under a 6-layer
masked PixelCNN (7x7 convs, 64 hidden channels, K=2 discrete values),
returning [B, 1].

Strategy (data-parallel over batch, 4 images per core, 8 cores):
  - Padded canvas layout per core: 4 images stacked vertically, each row
    stored with 3+3 zero pad cols (row stride S=70), 3 zero pad rows above
    each image.  All shifted conv reads then become flat offsets.
  - Activations h stored as [128, NPX]: partitions 0:64 = h, partitions
    64:128 = h shifted one row down (h2[p] = h[p+S]).  This gives K=128
    matmul contraction (2 kernel rows x 64 channels per instruction).
  - Each masked-conv matmul instruction covers a 2x2 kernel patch via a
    [128,128] lhsT:  A00=(ky,kx)  A10=(ky+1,kx)  A01=(ky,kx+1)
    A11=(ky+1,kx+1); the M halves accumulate into psum[0:64] / psum[64:128]
    and are combined as out[u] = psumA[u] + psumB[u+1].
  - Layer 1 (mask A, 1 input channel) uses a host-prepared 24-partition
    im2col canvas (one partition per mask-A tap).
  - Output head: d = (w_out[1]-w_out[0]) . h per pixel, accumulated into a
    [40, 490] psum (one partition per chunk via sliding-window lhsT);
    logp_pixel = -softplus(sigma * (d + db)), sigma = 1-2*sample (host
    prepared); reduce per chunk then per image with a tiny matmul.
"""
import os
import sys

import numpy as np

for _p in ("/opt/trn_rl_repo", "/root/.axon_site/_ro/trn_rl_repo"):
    if os.path.isdir(_p) and _p not in sys.path:
        sys.path.insert(0, _p)

import concourse.bass as bass
from concourse import bacc
import concourse.mybir as mybir
import concourse.tile as tile
from concourse.bass_utils import run_bass_kernel_spmd

# ---- problem constants (hardcoded; kernel.py must be self-contained) ----
B, C, H, W = 32, 1, 64, 64
KVALS = 2
HID = 64
KS = 7
NL = 6
NCORE = 8
BPC = B // NCORE        # images per core

S = 70                  # canvas row stride (3 + 64 + 3)
COL0 = 3
# per-image canvas tile: [1 margin row][3 pad rows][64 real rows] = 68 rows
ROW0 = 4 * S            # 280: canvas px of (row 0, col 0-incl-pad)
NPX = 68 * S            # 4760
MAXN = 7 * S            # 490

DT_MM = os.environ.get("PIXELCNN_DT", "f32r")   # f32r | bf16 | f32

# conv chunks within one image tile: 9 x 7-row + 1 x 1-row
CH = [(ROW0 + c * 7 * S, 7, 7 * S) for c in range(9)] + [(ROW0 + 63 * S, 1, S)]
# output head: uniform 4-row chunks, 16 per image; partition = 16*img + c
OUT_N = 4 * S           # 280
OUT_CH = [ROW0 + c * 4 * S for c in range(16)]


def _spatial_mask(kind):
    m = np.zeros((KS, KS), np.float32)
    c = KS // 2
    m[:c, :] = 1.0
    m[c, :c] = 1.0
    if kind == 'B':
        m[c, c] = 1.0
    return m


MASK_A = _spatial_mask('A')
MASK_B = _spatial_mask('B')
TAPS_A = [(ky, kx) for ky in range(KS) for kx in range(KS) if MASK_A[ky, kx]]
PATCHES_B = [(ky, kx) for ky in (0, 2) for kx in (0, 2, 4, 6)]


def _d_off(ky, kx):
    return (ky - 3) * S + (kx - 3)


# ---------------------------------------------------------------- host prep
def _pack_weights(ws, bs, w_out, b_out, np_dt):
    """Returns dict of host-packed weight arrays shared by all cores."""
    ws = [np.asarray(w, np.float32) for w in ws]
    bs = [np.asarray(b, np.float32) for b in bs]
    w_out = np.asarray(w_out, np.float32)[:, :, 0, 0]   # [2, HID]
    b_out = np.asarray(b_out, np.float32)
    dw = w_out[1] - w_out[0]
    db = float(b_out[1] - b_out[0])

    # mask-B layers: [128, 5*8*128] in SBUF layout [k, (li, patch, m)]
    wB = np.zeros((128, (NL - 1) * 8 * 128), np.float32)
    for li in range(NL - 1):
        w = ws[li + 1] * MASK_B[None, None]
        for pi, (ky, kx) in enumerate(PATCHES_B):
            col = (li * 8 + pi) * 128
            blk = np.zeros((128, 128), np.float32)
            for (dk, dm, kyy, kxx) in ((0, 0, ky, kx), (64, 0, ky + 1, kx),
                                       (0, 64, ky, kx + 1),
                                       (64, 64, ky + 1, kx + 1)):
                if kyy < KS and kxx < KS and MASK_B[kyy, kxx]:
                    blk[dk:dk + 64, dm:dm + 64] = w[:, :, kyy, kxx].T
            wB[:, col:col + 128] = blk

    w1 = np.stack([ws[0][:, 0, ky, kx] for (ky, kx) in TAPS_A], 0)  # [24, 64]

    wout = np.zeros((64, 129), np.float32)
    wout[:, 64] = dw

    # spl holds log(sigmoid(-x)) = -softplus(x) per pixel, so sum with +1
    wsel = np.zeros((64, 4), np.float32)
    for i in range(BPC):
        wsel[16 * i:16 * i + 16, i] = 1.0

    biasmat = np.stack(bs, 1)   # [64, NL]

    return {
        "wB": wB.astype(np_dt), "w1": w1.astype(np_dt),
        "wout": wout.astype(np_dt), "wsel": wsel,
        "biasmat": biasmat, "db": db,
    }


def _per_core_inputs(sample, np_dt):
    """sample [B,1,H,W] fp32 -> list of dicts with imc + sigma per core."""
    sample = np.asarray(sample, np.float32).reshape(B, H, W)
    maps = []
    for core in range(NCORE):
        smp = sample[core * BPC:(core + 1) * BPC]
        # per-image im2col canvases: part t = sample shifted by -d_t
        imc = np.zeros((BPC, 24, NPX + 300), np.float32)
        for t, (ky, kx) in enumerate(TAPS_A):
            d = _d_off(ky, kx)
            base = ROW0 + COL0 - d
            for i in range(BPC):
                for y in range(H):
                    imc[i, t, base + y * S: base + y * S + W] = smp[i, y]
        sigma = np.ones((64, OUT_N), np.float32)
        for p in range(64):
            i, c = p // 16, p % 16
            for r in range(4):
                sigma[p, r * S + COL0: r * S + COL0 + W] = \
                    1.0 - 2.0 * smp[i, 4 * c + r]
        maps.append({"imc": np.ascontiguousarray(imc[:, :, :NPX]).astype(np_dt),
                     "sigma": sigma})
    return maps


# ---------------------------------------------------------------- bass build
def _build_bass(db, mm_dt_name):
    f32 = mybir.dt.float32
    if mm_dt_name == "bf16":
        store_dt = mybir.dt.bfloat16
    elif mm_dt_name == "f32":
        store_dt = f32
    else:
        # float32r end-to-end: walrus requires producers of fp32r matmul
        # inputs to declare the f32r dtype (bitcasts fail BIR verification)
        store_dt = mybir.dt.float32r

    def mm(ap):
        return ap

    # Bacc (not raw Bass): its finalize() runs generate_event_semaphores,
    # which splits multi-wait sync onto event sems (HW allows 1 wait/inst)
    nc = bacc.Bacc("TRN2", target_bir_lowering=False)
    d_imc = nc.declare_dram_parameter("imc", [BPC, 24, NPX], store_dt, isOutput=False)
    d_wB = nc.declare_dram_parameter("wB", [128, (NL - 1) * 8 * 128], store_dt, isOutput=False)
    d_w1 = nc.declare_dram_parameter("w1", [24, 64], store_dt, isOutput=False)
    d_wout = nc.declare_dram_parameter("wout", [64, 129], store_dt, isOutput=False)
    d_wsel = nc.declare_dram_parameter("wsel", [64, 4], f32, isOutput=False)
    d_bias = nc.declare_dram_parameter("biasmat", [64, NL], f32, isOutput=False)
    d_sigma = nc.declare_dram_parameter("sigma", [64, OUT_N], f32, isOutput=False)
    d_out = nc.declare_dram_parameter("out", [BPC, 1], f32, isOutput=True)

    AF = mybir.ActivationFunctionType
    ALU = mybir.AluOpType
    AX = mybir.AxisListType

    with tile.TileContext(nc) as tc:
        with (
            tc.tile_pool(name="cpA", bufs=1) as cpA,
            tc.tile_pool(name="cpB", bufs=1) as cpB,
            tc.tile_pool(name="wp", bufs=1) as wp,
            tc.tile_pool(name="small", bufs=1) as sp_pool,
            tc.tile_pool(name="tmp", bufs=3) as tmp_pool,
            tc.tile_pool(name="psB", bufs=4, space="PSUM") as psB_pool,
            tc.tile_pool(name="ps1", bufs=2, space="PSUM") as ps1_pool,
            tc.tile_pool(name="psd", bufs=1, space="PSUM") as psd_pool,
        ):
            qeng = [nc.gpsimd, nc.sync, nc.scalar]

            # ---- layer-1 inputs first, so PE can start ASAP ----
            imc_t = []
            for i in range(BPC):
                t = cpA.tile([24, NPX], store_dt, tag=f"cA{i}")
                for q in range(3):
                    eng = qeng[(i * 3 + q) % 3]
                    eng.dma_start(t[8 * q:8 * q + 8, :], d_imc[i, 8 * q:8 * q + 8, :])
                imc_t.append(t)
            w1_t = wp.tile([24, 64], store_dt, tag="w1")
            nc.gpsimd.dma_start(w1_t[:, :], d_w1[:, :])
            bias_t = wp.tile([64, NL], f32, tag="bias")
            nc.gpsimd.dma_start(bias_t[:, :], d_bias[:, :])
            # h1 slots: pads must be zero before layer-1 combine writes
            m1 = []
            for i in range(BPC):
                t = cpB.tile([128, NPX], store_dt, tag=f"cB{i}")
                eng = nc.vector if i % 2 == 0 else nc.gpsimd
                eng.memset(t[:, :].bitcast(mybir.dt.uint32), 0)
                m1.append(t)

            # ---- remaining weights / consts (per-layer slices, spread) ----
            wB_t = wp.tile([128, (NL - 1) * 8 * 128], store_dt, tag="wB")
            for li in range(NL - 1):
                sl = bass.ts(li, 8 * 128)
                qeng[(1 + li) % 3].dma_start(wB_t[:, sl], d_wB[:, sl])
            wout_t = wp.tile([64, 129], store_dt, tag="wout")
            nc.sync.dma_start(wout_t[:, :], d_wout[:, :])
            wsel_t = wp.tile([64, 4], f32, tag="wsel")
            nc.sync.dma_start(wsel_t[:, :], d_wsel[:, :])
            sigma_t = wp.tile([64, OUT_N], f32, tag="sigma")
            nc.sync.dma_start(sigma_t[:, :], d_sigma[:, :])

            def views(t, parts, base, n_rows, shift):
                """[64, n_rows, 64] view of canvas t at pixel rows of a chunk."""
                lo = base + shift
                v = t[parts[0]:parts[1], lo:lo + n_rows * S]
                return v.rearrange("p (r c) -> p r c", c=S)[:, :, COL0:COL0 + W]

            def ps_view(ps, parts, n_rows, shift):
                v = ps[parts[0]:parts[1], shift:shift + n_rows * S]
                return v.rearrange("p (r c) -> p r c", c=S)[:, :, COL0:COL0 + W]

            # ---- layer 1 (mask A) ----
            h_cur = m1
            b0 = bias_t[:, 0:1]
            for i in range(BPC):
                for (cbase, n_rows, N) in CH:
                    ps = ps1_pool.tile([64, 512], f32, tag="ps1")
                    nc.tensor.matmul(ps[:, :N], mm(w1_t[:, :]),
                                     mm(imc_t[i][:, cbase:cbase + N]),
                                     start=True, stop=True)
                    nc.scalar.activation(views(h_cur[i], (0, 64), cbase, n_rows, 0),
                                         ps_view(ps, (0, 64), n_rows, 0),
                                         AF.Relu, bias=b0)
                    nc.vector.tensor_scalar(
                        views(h_cur[i], (64, 128), cbase, n_rows, -S),
                        ps_view(ps, (0, 64), n_rows, 0),
                        b0, 0.0, op0=ALU.add, op1=ALU.max)

            # ---- layers 2..NL (mask B) ----
            for li in range(NL - 1):
                pool, ctag = (cpA, "cA") if li % 2 == 0 else (cpB, "cB")
                h_nxt = []
                for i in range(BPC):
                    t = pool.tile([128, NPX], store_dt, tag=f"{ctag}{i}")
                    if li == 0:
                        # slot held im2col and was never fully zeroed: zero
                        # before combine writes (gpsimd, during layer-1 tail)
                        nc.gpsimd.memset(t[:, :].bitcast(mybir.dt.uint32), 0)
                    h_nxt.append(t)
                bl = bias_t[:, li + 1:li + 2]
                for i in range(BPC):
                    for (cbase, n_rows, N) in CH:
                        ps = psB_pool.tile([128, 512], f32, tag="psB")
                        for pi, (ky, kx) in enumerate(PATCHES_B):
                            o = cbase + _d_off(ky, kx)
                            nc.tensor.matmul(
                                ps[:, :N],
                                mm(wB_t[:, (li * 8 + pi) * 128:(li * 8 + pi) * 128 + 128]),
                                mm(h_cur[i][:, o:o + N]),
                                start=(pi == 0), stop=(pi == 7))
                        t1 = tmp_pool.tile([64, 7 * W], f32, tag="t1")
                        t1v = t1[:, :n_rows * W].rearrange("p (r c) -> p r c", c=W)
                        # t1 = psumA + bias  (ACT; one PSUM input per inst)
                        nc.scalar.activation(t1v, ps_view(ps, (0, 64), n_rows, 0),
                                             AF.Identity, bias=bl)
                        tmp = tmp_pool.tile([64, 7 * W], f32, tag="tmp")
                        tview = tmp[:, :n_rows * W].rearrange("p (r c) -> p r c", c=W)
                        # tmp = psumB_shifted + t1
                        nc.vector.scalar_tensor_tensor(
                            tview, ps_view(ps, (64, 128), n_rows, 1), 0.0, t1v,
                            op0=ALU.add, op1=ALU.add)
                        nc.scalar.activation(
                            views(h_nxt[i], (0, 64), cbase, n_rows, 0),
                            tview, AF.Relu)
                        nc.vector.tensor_scalar(
                            views(h_nxt[i], (64, 128), cbase, n_rows, -S),
                            tview, 0.0, None, op0=ALU.max)
                h_cur = h_nxt

            # ---- output head: d per 4-row chunk into psum [64, 512] ----
            dall = psd_pool.tile([64, 512], f32, tag="dall")
            for p in range(64):
                i, c = p // 16, p % 16
                cbase = OUT_CH[c]
                nc.tensor.matmul(dall[:, :OUT_N], mm(wout_t[:, 64 - p:128 - p]),
                                 mm(h_cur[i][0:64, cbase:cbase + OUT_N]),
                                 start=(p == 0), stop=(p == 63))

            sd_t = sp_pool.tile([64, OUT_N], f32, tag="sd")
            nc.vector.scalar_tensor_tensor(sd_t[:, :], dall[:, :OUT_N], float(db),
                                           sigma_t[:, :], op0=ALU.add,
                                           op1=ALU.mult)
            # -softplus(x) = log(sigmoid(-x)); Softplus has no ACT func set
            sg_t = sp_pool.tile([64, OUT_N], f32, tag="sg")
            nc.scalar.activation(sg_t[:, :], sd_t[:, :], AF.Sigmoid, scale=-1.0)
            spl_t = sp_pool.tile([64, OUT_N], f32, tag="spl")
            nc.scalar.activation(spl_t[:, :], sg_t[:, :], AF.Ln)

            sums_t = sp_pool.tile([64, 1], f32, tag="sums")
            allv = spl_t[:, :].rearrange("p (r c) -> p r c", c=S)[:, :, COL0:COL0 + W]
            nc.vector.tensor_reduce(sums_t[:, 0:1], allv, axis=AX.XY, op=ALU.add)

            fin = psd_pool.tile([4, 1], f32, tag="fin")
            nc.tensor.matmul(fin[0:4, 0:1], wsel_t[:, :], sums_t[:, 0:1],
                             start=True, stop=True)
            out_sb = sp_pool.tile([4, 1], f32, tag="outsb")
            nc.scalar.copy(out_sb[:, :], fin[0:4, 0:1])
            nc.gpsimd.dma_start(d_out[:, :], out_sb[:, :])

    return nc


_CACHE = {}


def _get_nc(db):
    key = (DT_MM, round(db, 9))
    if key not in _CACHE:
        nc = _build_bass(db, DT_MM)
        nc.finalize()   # Bacc.compile(): event-sem split, reg alloc, codegen
        _CACHE[key] = nc
    return _CACHE[key]


def kernel(sample, ws, bs, w_out, b_out):
    np_dt = np.float32
    if DT_MM == "bf16":
        import ml_dtypes
        np_dt = ml_dtypes.bfloat16

    packed = _pack_weights(ws, bs, w_out, b_out, np_dt)
    core_maps = _per_core_inputs(sample, np_dt)
    shared = {k: packed[k] for k in ("wB", "w1", "wout", "wsel", "biasmat")}
    in_maps = [dict(shared, **m) for m in core_maps]

    nc = _get_nc(packed["db"])
    trace = bool(int(os.environ.get("PIXELCNN_TRACE", "0")))
    res = run_bass_kernel_spmd(nc, in_maps, list(range(NCORE)), trace=trace)
    if trace and res.exec_time_ns is not None:
        print(f"HW exec time: {res.exec_time_ns} ns")
        if res.mean_exec_time_ns is not None:
            print(f"HW mean exec time: {res.mean_exec_time_ns} ns")
    out = np.concatenate([np.asarray(r["out"], np.float32) for r in res.results], 0)
    return out
